# revision 6
# baseline (speedup 1.0000x reference)
"""FAST multi-head attention (p=2 Taylor linear attention) for Trainium2.

Self-contained: accepts FULL inputs q,k,v [2,16,4096,32] fp32, returns the
full output [2,16,4096,32]. Shards the 32 (b,h) pairs across 8 NeuronCores
(4 per core), one Bass/Tile kernel run SPMD via PJRT.

Per (b,h) (A0=1, A1=1, A2=0.5):
  num[n,e'] = sum_m v'[m,e'] * (A0 + A1 (q.k_m) + A2 (q.k_m)^2),  v' = [v | 1]
  out       = num[:, :32] / num[:, 32]
factorized through quadratic features with a cyclic pair cover
(gap = 16..1, descending):
  k-side:  ktile[m, 33+32p+d] = k_d * k_{(d+16-p)%32}  (one DVE op per
           128-row tile, negative-stride output), diag k^2 at cols 545:577
  kv:      kvt_A = v'^T [ones|k|cover_hi], kvt_B = v'^T [cover_lo|diag] (PE)
  q-side:  PhiOff^T = Square(E^T qT) (PE + ScalarE), PhiD^T = A2 qT^2
  ansT    = wc^T PhiOff^T + wd^T PhiD^T + wq^T qT + bias  (PE, fp32r)
The device stores ansT [33, N] (numerator rows 0:32, denominator row 32);
the divide + final [e,n]->[n,e] transpose happen on the host.
Spurious diagonal terms from the cover are cancelled through the wd weights
(wd = KV2dd - Hmat @ KVc).
"""
import dataclasses
import numpy as np

import concourse.bass as bass
import concourse.tile as tile
from concourse import mybir, bacc
from concourse.bass_utils import run_bass_kernel_spmd

F32 = mybir.dt.float32
A0, A1, A2 = 1.0, 1.0, 0.5
B, H, N, D = 2, 16, 4096, 32
NJ = 16                    # cover gaps, stored descending 16..1
F = NJ * D                 # 512 off-diagonal features
NCORES = 8
BH_PER_CORE = (B * H) // NCORES   # 4
NT = N // 128              # 32 n-tiles per (b,h)
E1 = D + 2                 # 34 output rows: 32 num + den + dup-den pad


def _host_consts():
    E = np.zeros((D, F), np.float32)
    Hm = np.zeros((D, F), np.float32)
    for jj in range(NJ):          # block jj holds gap = 16 - jj
        gap = NJ - jj
        beta2 = A2 if gap < 16 else A2 / 2.0
        beta = np.sqrt(beta2).astype(np.float32)
        c = beta2 / A2
        for d1 in range(D):
            f = jj * D + d1
            d2 = (d1 + gap) % D
            E[d1, f] += beta
            E[d2, f] += beta
            Hm[d1, f] += c
            Hm[d2, f] += c
    E4 = np.tile(E, (4, 1)).astype(np.float32)               # [128, 512]
    HmT = Hm.T.reshape(4, 128, D).transpose(1, 0, 2).copy()  # [128, 4, 32]
    ident = np.eye(128, dtype=np.float32)
    return E4, HmT, ident


def _ap_free(x: bass.AP, free_ap, extra_offset=0):
    return dataclasses.replace(
        x, offset=x.offset + extra_offset, ap=[x.ap[0]] + [list(p) for p in free_ap]
    )


def build_nc():
    nc = bacc.Bacc(None, target_bir_lowering=False)
    R32 = mybir.dt.float32r

    def r(ap):
        return ap if ap.dtype == R32 else ap.bitcast(R32)

    def tr(out_ap, in_ap, ident_ap, tile_position=None):
        nc.tensor.matmul(out_ap, in_ap, ident_ap, is_transpose=True,
                         tile_position=tile_position, skip_group_check=True)

    qin = nc.declare_dram_parameter("qin", [BH_PER_CORE, N, D], F32, isOutput=False)
    kin = nc.declare_dram_parameter("kin", [BH_PER_CORE, N, D], F32, isOutput=False)
    vin = nc.declare_dram_parameter("vin", [BH_PER_CORE, N, D], F32, isOutput=False)
    e4_in = nc.declare_dram_parameter("e4", [128, F], F32, isOutput=False)
    hmt_in = nc.declare_dram_parameter("hmt", [128, 4, D], F32, isOutput=False)
    id_in = nc.declare_dram_parameter("ident", [128, 128], F32, isOutput=False)
    out = nc.declare_dram_parameter("out", [BH_PER_CORE, E1, N], F32, isOutput=True)

    SQ = mybir.ActivationFunctionType.Square
    sqrt_a2 = float(np.sqrt(A2))

    with tile.TileContext(nc) as tc:
        with (
            tc.tile_pool(name="sb_const", bufs=1) as sb_const,
            tc.tile_pool(name="sb_q", bufs=2) as sb_q,
            tc.tile_pool(name="sb_k", bufs=3) as sb_k,
            tc.tile_pool(name="sb_w", bufs=2) as sb_w,
            tc.tile_pool(name="sb_phi", bufs=3) as sb_phi,
            tc.tile_pool(name="sb_ep", bufs=3) as sb_ep,
            tc.tile_pool(name="ps_kv", bufs=1, space="PSUM") as ps_kv,
            tc.tile_pool(name="ps_u", bufs=2, space="PSUM") as ps_u,
            tc.tile_pool(name="ps_ans", bufs=2, space="PSUM") as ps_ans,
            tc.tile_pool(name="ps_sm", bufs=2, space="PSUM") as ps_sm,
        ):
            e4 = sb_const.tile([128, F], R32)
            nc.sync.dma_start(out=e4[:], in_=e4_in[:].bitcast(R32))
            hmt = sb_const.tile([128, 4, D], R32)
            nc.sync.dma_start(out=hmt[:], in_=hmt_in[:].bitcast(R32))
            ident = sb_const.tile([128, 128], F32)
            nc.sync.dma_start(out=ident[:], in_=id_in[:])

            for b in range(BH_PER_CORE):
                qv = qin[b].rearrange("(a bb p) d -> p bb a d", a=4, bb=8)
                kv_ = kin[b].rearrange("(t p) d -> p t d", p=128)
                vv = vin[b].rearrange("(t p) d -> p t d", p=128)

                # ---------- q loads + transposes ------------------------------
                q_sb = sb_q.tile([128, 8, 4, D], F32, tag="q_sb")
                for a in range(4):
                    nc.sync.dma_start(out=q_sb[:, :, a, :], in_=qv[:, :, a, :])
                qtb = sb_q.tile([128, 8, 128], R32, tag="qtb")
                for bb in range(8):
                    qt_ps = ps_sm.tile([128, 128], F32, tag="sm")
                    tr(qt_ps[:], q_sb[:, bb, :, :], ident[:])
                    nc.scalar.copy(out=qtb[:, bb, :], in_=qt_ps[:])
                phidt = sb_q.tile([128, 8, 128], R32, tag="phidt")
                nc.scalar.activation(out=phidt[:], in_=qtb[:], func=SQ,
                                     scale=sqrt_a2)

                # ---------- phase 1: k features + KV accumulation -------------
                kvt_a = ps_kv.tile([E1, 290], F32, tag="kvt_a")
                kvt_b = ps_kv.tile([E1, 288], F32, tag="kvt_b")
                for g in range(4):
                    ts8 = slice(8 * g, 8 * g + 8)
                    stg = sb_k.tile([128, 8, 48], F32, tag="stg")
                    nc.sync.dma_start(out=stg[:, :, 0:32], in_=kv_[:, ts8, :])
                    nc.sync.dma_start(out=stg[:, :, 32:48],
                                      in_=kv_[:, ts8, 0:16])
                    kt = sb_k.tile([128, 8, 578], R32, tag="kt")
                    nc.gpsimd.memset(kt[:, :, 0:2].bitcast(F32), 1.0)
                    nc.sync.dma_start(out=kt[:, :, 2:34],
                                      in_=kv_[:, ts8, :].bitcast(R32))
                    vx = sb_k.tile([128, 8, 34], R32, tag="vx")
                    nc.sync.dma_start(out=vx[:, :, 0:32],
                                      in_=vv[:, ts8, :].bitcast(R32))
                    nc.gpsimd.memset(vx[:, :, 32:34].bitcast(F32), 1.0)
                    for tt in range(8):
                        t = 8 * g + tt
                        kbase = stg[:, tt, 0:32]
                        in0 = _ap_free(kbase, [[0, NJ + 1], [1, D]])
                        in1 = _ap_free(kbase, [[1, NJ + 1], [1, D]])
                        # out col for gap j at 545-32j (diag j=0 at 545:577)
                        dst = _ap_free(kt[:, tt, 546:547],
                                       [[-D, NJ + 1], [1, D]])
                        nc.vector.tensor_mul(dst, in0, in1)
                        lhs = r(vx[:, tt, :])
                        st, sp = (t == 0), (t == NT - 1)
                        nc.tensor.matmul(kvt_a[:], lhs, r(kt[:, tt, 0:290]),
                                         start=st, stop=sp)
                        nc.tensor.matmul(kvt_b[:], lhs, r(kt[:, tt, 290:578]),
                                         start=st, stop=sp)

                # ---------- phase 2: weight assembly --------------------------
                a_sb = sb_w.tile([E1, 290], F32, tag="a_sb")
                nc.vector.tensor_copy(a_sb[:], kvt_a[:])
                b_sb = sb_w.tile([E1, 288], F32, tag="b_sb")
                nc.vector.tensor_copy(b_sb[:], kvt_b[:])

                # wc: cover features (gap 16..1), 4 groups of 128
                wc = sb_w.tile([128, 4, E1], R32, tag="wc")
                cov = [a_sb[:, 34:162], a_sb[:, 162:290],
                       b_sb[:, 0:128], b_sb[:, 128:256]]
                for s in range(4):
                    trc = ps_sm.tile([128, E1], F32, tag="sm")
                    tr(trc[0:128, :], cov[s], ident[0:E1, 0:E1])
                    nc.scalar.copy(out=wc[:, s, :], in_=trc[0:128, :])

                # HKVcT [E1, 32] = (Hmat @ KVc)^T
                hk = ps_sm.tile([E1, D], F32, tag="sm")
                for s in range(4):
                    nc.tensor.matmul(hk[:], r(wc[:, s, :]), r(hmt[:, s, :]),
                                     start=(s == 0), stop=(s == 3))

                # wDT [E1, 32] = KV2ddT - HKVcT
                wdt = sb_w.tile([E1, D], F32, tag="wdt")
                nc.vector.scalar_tensor_tensor(
                    out=wdt[:], in0=b_sb[:, 256:288], scalar=1.0, in1=hk[:],
                    op0=mybir.AluOpType.mult, op1=mybir.AluOpType.subtract,
                )

                # wq4/wd4: transpose to [32, E1], replicate to 4 groups
                wq4 = sb_w.tile([128, E1], R32, tag="wq4")
                wd4 = sb_w.tile([128, E1], R32, tag="wd4")
                trq = ps_sm.tile([128, E1], F32, tag="sm")
                trd = ps_sm.tile([128, E1], F32, tag="sm")
                tr(trq[0:32, :], a_sb[:, 2:34], ident[0:E1, 0:E1])
                tr(trd[0:32, :], wdt[:], ident[0:E1, 0:E1])
                nc.scalar.copy(out=wq4[0:32, :], in_=trq[0:32, :])
                nc.scalar.copy(out=wd4[0:32, :], in_=trd[0:32, :])
                for a in range(1, 4):
                    nc.sync.dma_start(out=wq4[32 * a:32 * a + 32, :],
                                      in_=wq4[0:32, :])
                    nc.sync.dma_start(out=wd4[32 * a:32 * a + 32, :],
                                      in_=wd4[0:32, :])

                # ---------- phase 3: paired 512-wide n-chunks -----------------
                for half in range(2):
                    blk = slice(4 * half, 4 * half + 4)
                    for g2 in range(2):
                        pair = (g2, g2 + 2)
                        phit = [sb_phi.tile([128, 4, 512], R32, tag="phit",
                                            name=f"phit{ci}")
                                for ci in range(2)]
                        for s in range(4):
                            for ci, a in enumerate(pair):
                                pa = slice(32 * a, 32 * a + 32)
                                u_ps = ps_u.tile([128, 512], F32, tag="u")
                                nc.tensor.matmul(
                                    u_ps[:],
                                    r(e4[pa, 128 * s:128 * (s + 1)]),
                                    r(qtb[pa, blk, :]),
                                    tile_position=(32 * a, 0))
                                nc.scalar.activation(
                                    out=phit[ci][:, s, :], in_=u_ps[:],
                                    func=SQ, scale=1.0)
                        for ci, a in enumerate(pair):
                            pa = slice(32 * a, 32 * a + 32)
                            ansT = ps_ans.tile([E1, 512], F32, tag="ansT")
                            for s in range(4):
                                nc.tensor.matmul(ansT[:], r(wc[:, s, :]),
                                                 r(phit[ci][:, s, :]),
                                                 start=(s == 0), stop=False)
                            nc.tensor.matmul(ansT[:], r(wd4[pa, :]),
                                             r(phidt[pa, blk, :]),
                                             start=False, stop=False,
                                             tile_position=(32 * a, 0))
                            nc.tensor.matmul(ansT[:], r(wq4[pa, :]),
                                             r(qtb[pa, blk, :]),
                                             start=False, stop=True,
                                             tile_position=(32 * a, 0))
                            anssb = sb_ep.tile([E1, 512], F32, tag="anssb")
                            nc.vector.tensor_scalar_add(anssb[:], ansT[:],
                                                        a_sb[:, 0:1])
                            off = 1024 * a + 512 * half
                            nc.sync.dma_start(out=out[b][:, off:off + 512],
                                              in_=anssb[:])

    nc.compile()
    return nc


_NC_CACHE = None


def _get_nc():
    global _NC_CACHE
    if _NC_CACHE is None:
        _NC_CACHE = build_nc()
    return _NC_CACHE


def _in_maps(q, k, v):
    qf = q.reshape(B * H, N, D)
    kf = k.reshape(B * H, N, D)
    vf = v.reshape(B * H, N, D)
    E4, HmT, ident = _host_consts()
    in_maps = []
    for c in range(NCORES):
        sl = slice(c * BH_PER_CORE, (c + 1) * BH_PER_CORE)
        in_maps.append({
            "qin": np.ascontiguousarray(qf[sl]),
            "kin": np.ascontiguousarray(kf[sl]),
            "vin": np.ascontiguousarray(vf[sl]),
            "e4": E4, "hmt": HmT, "ident": ident,
        })
    return in_maps


def _postprocess(res):
    outs = [res.results[c]["out"] for c in range(NCORES)]
    allt = np.concatenate(outs, axis=0).reshape(B, H, E1, N)
    num = allt[:, :, 0:D, :]
    den = allt[:, :, D:D + 1, :]
    return np.ascontiguousarray(
        (num / den).transpose(0, 1, 3, 2)).astype(np.float32)


def run_traced(q, k, v):
    q = np.ascontiguousarray(np.asarray(q, dtype=np.float32))
    k = np.ascontiguousarray(np.asarray(k, dtype=np.float32))
    v = np.ascontiguousarray(np.asarray(v, dtype=np.float32))
    nc = _get_nc()
    try:
        return run_bass_kernel_spmd(nc, _in_maps(q, k, v),
                                    core_ids=list(range(NCORES)), trace=True)
    except Exception as e:
        print("traced run failed:", e)
        return None


def kernel(q, k, v):
    q = np.ascontiguousarray(np.asarray(q, dtype=np.float32))
    k = np.ascontiguousarray(np.asarray(k, dtype=np.float32))
    v = np.ascontiguousarray(np.asarray(v, dtype=np.float32))
    assert q.shape == (B, H, N, D)
    nc = _get_nc()
    res = run_bass_kernel_spmd(nc, _in_maps(q, k, v),
                               core_ids=list(range(NCORES)))
    return _postprocess(res)


if __name__ == "__main__":
    rng = np.random.default_rng(0)
    q = rng.standard_normal((B, H, N, D), dtype=np.float32)
    k = rng.standard_normal((B, H, N, D), dtype=np.float32)
    v = rng.standard_normal((B, H, N, D), dtype=np.float32)
    o = kernel(q, k, v)
    print("ran", o.shape, o.dtype)
